# revision 10
# baseline (speedup 1.0000x reference)
"""Trainium2 Bass kernel for nn_DecoderLSTM_B (B=32,S=256,V=32000,E=H=128).

Strategy: data-parallel over batch across 8 cores (4 batches/core).
Per core:
  - host pre-gathers embeddings (transposed, t-major tokens) and pre-folds
    LSTM biases (b_ih + h0@W_hh.T + b_hh), with the cell-gate block
    pre-scaled by 2 so tanh(x) = 2*sigmoid(2x)-1 needs only sigmoid LUTs.
  - device: Xproj = W_x @ x for all tokens (PE), then the 256-step LSTM
    recurrence in [H, B] layout (4 small matmuls + 2 sigmoids + 6 DVE ops
    per step), then a two-pass float32r logits pipeline:
      pass 1 (v-on-partitions): logits tiles -> exp (ACT) -> sum over v via
        ones/e^{b_pred}-weighted stationary matmuls accumulating in PSUM ->
        log-sum-exp per token (no max subtraction: |logits| <= ~8 bounded).
      pass 2 (tokens-on-partitions): recompute logits, add bias via a
        rank-1 PE matmul-materialized tile, evict with one fused DVE
        scalar_tensor_tensor: out = (logits + (-LSE)) + b_pred.
  - output [1024, 32000] f32 per core, host reassembles to [32,256,32000].
"""
import sys
sys.path.insert(0, '/opt/trn_rl_repo')

import numpy as np
from contextlib import ExitStack

B, S, V, E, H = 32, 256, 32000, 128, 128
NCORES = 8
BL = B // NCORES            # 4 batches per core
TOK = BL * S                # 1024 tokens per core (t-major: tok = t*BL + b)
NBLK = TOK // 128           # 8 token blocks of 128
G = 2                       # pass-1 token groups of 512
GTOK = TOK // G
VT1 = 128                   # pass-1 vocab tile (stationary M)
NVT1 = V // VT1             # 250
CH = 2                      # pass-1 vtiles per psum chunk (2 banks)
NCHUNK = NVT1 // CH         # 125
V_TILES2 = [(i * 512, 512) for i in range(V // 512)]
if V % 512:
    V_TILES2.append((V - V % 512, V % 512))           # 62x512 + 1x256

_PROGRAM = None


def _build_program():
    from concourse import bass, tile, mybir, bacc
    F32 = mybir.dt.float32
    F32R = mybir.dt.float32r
    AF = mybir.ActivationFunctionType
    ALU = mybir.AluOpType

    nc = bacc.Bacc("TRN2", target_bir_lowering=False, debug=False,
                   num_devices=NCORES)

    # ---- DRAM I/O (per-core shapes) ----
    xT_d = nc.dram_tensor("xT", [E, TOK], F32, kind="ExternalInput").ap()
    xbias_d = nc.dram_tensor("xbias", [128, 4 * 512], F32, kind="ExternalInput").ap()
    whT_d = nc.dram_tensor("whT", [H, 4 * 128], F32, kind="ExternalInput").ap()
    wxT_d = nc.dram_tensor("wxT", [E, 4 * 128], F32, kind="ExternalInput").ap()
    c0T_d = nc.dram_tensor("c0T", [H, BL], F32, kind="ExternalInput").ap()
    wpredT_d = nc.dram_tensor("wpredT", [H, V], F32R, kind="ExternalInput").ap()
    ebT_d = nc.dram_tensor("ebT", [128, NVT1], F32R, kind="ExternalInput").ap()
    bpred_d = nc.dram_tensor("bpred", [1, V], F32R, kind="ExternalInput").ap()
    out_d = nc.dram_tensor("out", [TOK, V], F32, kind="ExternalOutput").ap()

    with tile.TileContext(nc) as tc:
        with ExitStack() as ctx:
            cst = ctx.enter_context(tc.tile_pool(name="cst", bufs=1))
            big = ctx.enter_context(tc.tile_pool(name="big", bufs=1))
            wrk = ctx.enter_context(tc.tile_pool(name="wrk", bufs=2))

            # ---- preload small constants ----
            xT = cst.tile([E, TOK], F32)
            nc.sync.dma_start(xT[:], xT_d[:])
            xbias = cst.tile([128, 4 * 512], F32)
            nc.sync.dma_start(xbias[:], xbias_d[:])
            whT = cst.tile([H, 4 * 128], F32)
            nc.sync.dma_start(whT[:], whT_d[:])
            wxT = cst.tile([E, 4 * 128], F32)
            nc.sync.dma_start(wxT[:], wxT_d[:])
            c0T = cst.tile([H, BL], F32)
            nc.sync.dma_start(c0T[:], c0T_d[:])
            ebT = cst.tile([128, NVT1], F32R)
            nc.sync.dma_start(ebT[:], ebT_d[:])

            ones_f = cst.tile([1, 128], F32)
            nc.vector.memset(ones_f[:], 1.0)
            ones_r = cst.tile([1, 128], F32R)
            nc.vector.tensor_copy(ones_r[:], ones_f[:])
            ident = cst.tile([1, 1], F32)
            nc.vector.memset(ident[:], 1.0)

            tc.strict_bb_all_engine_barrier()

            # big W_pred.T load issued post-barrier (only pass-1/2 need it)
            wpredT = big.tile([H, V], F32R)
            nc.sync.dma_start(wpredT[:], wpredT_d[:])

            # ---- persistent state buffers ----
            xbuf = cst.tile([128, S * 16], F32)        # [j, t, g, b]
            hsT = cst.tile([H, TOK], F32)              # [h, tok] t-major
            hsT_r = cst.tile([H, TOK], F32R)

            xbuf_v = xbuf[:].rearrange("p (t g b) -> p t g b", t=S, g=4, b=BL)

            # ---- phase 0: Xproj (x @ W_x.T per gate) + bias fold ----
            with tc.tile_pool(name="xp_ps", bufs=2, space="PSUM") as xp_ps:
                for gate in range(4):
                    for tchunk in range(TOK // 512):
                        pt = xp_ps.tile([128, 512], F32, tag="xp")
                        nc.tensor.matmul(
                            pt[:], wxT[:, gate * 128:(gate + 1) * 128],
                            xT[:, tchunk * 512:(tchunk + 1) * 512],
                            start=True, stop=True)
                        # evict+bias: xbuf[:, trange, gate, :] = psum + bias_rep
                        dst = xbuf_v[:, tchunk * 128:(tchunk + 1) * 128, gate, :]
                        src = pt[:].rearrange("p (t b) -> p t b", b=BL)
                        bias = xbias[:, gate * 512:(gate + 1) * 512].rearrange(
                            "p (t b) -> p t b", b=BL)
                        nc.vector.tensor_tensor(
                            out=dst, in0=src, in1=bias, op=ALU.add)

            # ---- phase 1: LSTM recurrence (t-major; state in [H, BL]) ----
            with tc.tile_pool(name="g_ps", bufs=2, space="PSUM") as g_ps, \
                 tc.tile_pool(name="lst", bufs=3) as lst:
                for t in range(S):
                    if t == 0:
                        gates_sb = xbuf[:, 0:16]
                    else:
                        gp = g_ps.tile([128, 16], F32, tag="g")
                        hprev = hsT[:, (t - 1) * BL:t * BL]
                        for gate in range(4):
                            nc.tensor.matmul(
                                gp[:, gate * BL:(gate + 1) * BL],
                                whT[:, gate * 128:(gate + 1) * 128],
                                hprev, start=True, stop=True)
                        gates_sb = lst.tile([128, 16], F32, tag="gsb")
                        nc.vector.tensor_tensor(
                            out=gates_sb[:], in0=gp[:],
                            in1=xbuf[:, t * 16:(t + 1) * 16], op=ALU.add)
                    sig = lst.tile([128, 16], F32, tag="sig")
                    nc.scalar.activation(sig[:], gates_sb[:], AF.Sigmoid,
                                         bias=0.0, scale=1.0)
                    si = sig[:, 0:BL]
                    sf = sig[:, BL:2 * BL]
                    sg2 = sig[:, 2 * BL:3 * BL]
                    so = sig[:, 3 * BL:4 * BL]
                    t1 = lst.tile([128, BL], F32, tag="t1")
                    nc.vector.tensor_tensor(out=t1[:], in0=sf, in1=c0T[:], op=ALU.mult)
                    a = lst.tile([128, BL], F32, tag="a")
                    nc.vector.tensor_tensor(out=a[:], in0=sg2, in1=si, op=ALU.mult)
                    u = lst.tile([128, BL], F32, tag="u")
                    nc.vector.scalar_tensor_tensor(
                        out=u[:], in0=a[:], scalar=2.0, in1=si,
                        op0=ALU.mult, op1=ALU.subtract)
                    c = lst.tile([128, BL], F32, tag="c")
                    nc.vector.tensor_tensor(out=c[:], in0=t1[:], in1=u[:], op=ALU.add)
                    sc = lst.tile([128, BL], F32, tag="sc")
                    nc.scalar.activation(sc[:], c[:], AF.Sigmoid,
                                         bias=0.0, scale=2.0)
                    b2 = lst.tile([128, BL], F32, tag="b2")
                    nc.vector.tensor_tensor(out=b2[:], in0=sc[:], in1=so, op=ALU.mult)
                    nc.vector.scalar_tensor_tensor(
                        out=hsT[:, t * BL:(t + 1) * BL], in0=b2[:], scalar=2.0,
                        in1=so, op0=ALU.mult, op1=ALU.subtract)

            # cast hs to f32r for the logits matmuls
            for k in range(2):
                nc.vector.tensor_copy(hsT_r[:, k * 512:(k + 1) * 512],
                                      hsT[:, k * 512:(k + 1) * 512])

            # ---- phase 2: per 512-token group: pass1 (LSE) then pass2 ----
            p1_ps = ctx.enter_context(
                tc.tile_pool(name="p1_ps", bufs=2, space="PSUM"))
            sum_ps = ctx.enter_context(
                tc.tile_pool(name="sum_ps", bufs=1, space="PSUM"))
            p2_ps = ctx.enter_context(
                tc.tile_pool(name="p2_ps", bufs=2, space="PSUM"))
            msc_ps = ctx.enter_context(
                tc.tile_pool(name="msc_ps", bufs=1, space="PSUM"))

            neglse_cols = []   # [128,1] per 128-token block
            for g in range(G):
                toks = slice(g * GTOK, (g + 1) * GTOK)
                hs_g = hsT_r[:, toks]
                # pass 1: sums[tok] = sum_v e^{b_v} * exp(logit[v, tok])
                sums = sum_ps.tile([1, GTOK], F32, tag="sums")
                for chunk in range(NCHUNK):
                    pc = p1_ps.tile([128, CH * 512], F32, tag="p1c")
                    for q in range(CH):
                        vt = chunk * CH + q
                        nc.tensor.matmul(
                            pc[:, q * 512:(q + 1) * 512],
                            wpredT[:, vt * 128:(vt + 1) * 128],
                            hs_g, start=True, stop=True)
                    ex = wrk.tile([128, CH * 512], F32R, tag="ex")
                    nc.scalar.activation(ex[:], pc[:], AF.Exp,
                                         bias=0.0, scale=1.0)
                    for q in range(CH):
                        vt = chunk * CH + q
                        nc.tensor.matmul(
                            sums[:], ebT[:, vt:vt + 1],
                            ex[:, q * 512:(q + 1) * 512],
                            start=(vt == 0), stop=(vt == NVT1 - 1),
                            skip_group_check=True)
                # LSE row, negated, transposed to per-block [128,1] columns
                lse_row = wrk.tile([1, GTOK], F32, tag="lse")
                nc.scalar.activation(lse_row[:], sums[:], AF.Ln,
                                     bias=0.0, scale=1.0)
                neg_row = wrk.tile([1, GTOK], F32, tag="neg")
                nc.vector.tensor_scalar_mul(neg_row[:], lse_row[:], -1.0)
                for j in range(GTOK // 128):
                    tp = msc_ps.tile([128, 1], F32, tag="msc")
                    nc.tensor.transpose(tp[:], neg_row[:, j * 128:(j + 1) * 128],
                                        ident[:])
                    col = cst.tile([128, 1], F32, tag=f"nlse{g}_{j}")
                    nc.vector.tensor_copy(col[:], tp[:])
                    neglse_cols.append(col)

                # pass 2: out = (logits + (-LSE)) + b_pred
                for (vo, vw) in V_TILES2:
                    brow = wrk.tile([1, 512], F32R, tag="brow")
                    nc.sync.dma_start(brow[:1, :vw], bpred_d[:, vo:vo + vw])
                    bp = msc_ps.tile([128, 512], F32, tag="msc")
                    nc.tensor.matmul(bp[:, :vw], ones_r[:],
                                     brow[:1, :vw], start=True, stop=True)
                    bias_sb = wrk.tile([128, 512], F32, tag="bias_sb")
                    nc.vector.tensor_copy(bias_sb[:, :vw], bp[:, :vw])
                    for jb in range(GTOK // 128):
                        blk = g * (GTOK // 128) + jb
                        pt2 = p2_ps.tile([128, 512], F32, tag="p2t")
                        nc.tensor.matmul(
                            pt2[:, :vw], hsT_r[:, blk * 128:(blk + 1) * 128],
                            wpredT[:, vo:vo + vw],
                            start=True, stop=True)
                        osb = wrk.tile([128, 512], F32, tag="osb")
                        nc.vector.scalar_tensor_tensor(
                            out=osb[:, :vw], in0=pt2[:, :vw],
                            scalar=neglse_cols[blk][:],
                            in1=bias_sb[:, :vw], op0=ALU.add, op1=ALU.add)
                        nc.sync.dma_start(
                            out_d[blk * 128:(blk + 1) * 128, vo:vo + vw],
                            osb[:, :vw])

    nc.compile()
    return nc


def _get_program():
    global _PROGRAM
    if _PROGRAM is None:
        _PROGRAM = _build_program()
    return _PROGRAM


def kernel(sequence, encoder_output, encoder_output_hidden, encoder_output_cell,
           emb, W_ih, b_ih, W_hh, b_hh, W_pred, b_pred):
    from concourse import bass_utils

    seq = np.asarray(sequence)
    emb = np.asarray(emb, dtype=np.float32)
    W_ih = np.asarray(W_ih, dtype=np.float32)
    b_ih = np.asarray(b_ih, dtype=np.float32)
    W_hh = np.asarray(W_hh, dtype=np.float32)
    b_hh = np.asarray(b_hh, dtype=np.float32)
    W_pred = np.asarray(W_pred, dtype=np.float32)
    b_pred = np.asarray(b_pred, dtype=np.float32)
    h0 = np.asarray(encoder_output_hidden, dtype=np.float32)[0]   # [B, H]
    c0 = np.asarray(encoder_output_cell, dtype=np.float32)[0]     # [B, H]

    # host prep ------------------------------------------------------------
    W_x = W_ih[:, :E].copy()          # [4H, E]
    W_h = W_ih[:, E:].copy()          # [4H, H]
    hh = h0 @ W_hh.T + b_hh           # [B, 4H]
    bias_full = b_ih[None, :] + hh    # [B, 4H]
    # pre-scale cell gate (block 2 of 4) by 2 for the tanh-via-sigmoid trick
    W_x[2 * H:3 * H, :] *= 2.0
    W_h[2 * H:3 * H, :] *= 2.0
    bias_full = bias_full.copy()
    bias_full[:, 2 * H:3 * H] *= 2.0

    whT = np.ascontiguousarray(W_h.T).reshape(H, 4 * 128)   # [h, (g j)] -> per gate cols
    # note: W_h.T is [H, 4H]; gate g block = cols g*128:(g+1)*128  (matches kernel)
    wxT = np.ascontiguousarray(W_x.T).reshape(E, 4 * 128)
    wpredT = np.ascontiguousarray(W_pred.T)                 # [H, V]
    ebT = np.exp(b_pred).astype(np.float32).reshape(NVT1, VT1).T.copy()  # [128, 250]
    bpred_row = b_pred.reshape(1, V).astype(np.float32)

    x_all = emb[seq]                                        # [B, S, E]

    in_maps = []
    for core in range(NCORES):
        bs = slice(core * BL, (core + 1) * BL)
        # xT: [E, TOK] t-major (tok = t*BL + b)
        xc = x_all[bs]                                      # [BL, S, E]
        xT = np.ascontiguousarray(xc.transpose(2, 1, 0)).reshape(E, TOK)
        # xbias: [128, 4*512]: per gate [j, (t128? no: (t? ...))]
        # bias_rep_g[j, t, b] = bias_full[b, g*128+j], tiled over t (128)
        bf = bias_full[bs]                                  # [BL, 4H]
        xb = np.empty((128, 4, 128, BL), dtype=np.float32)
        for gate in range(4):
            blkv = bf[:, gate * 128:(gate + 1) * 128].T     # [128j, BL]
            xb[:, gate, :, :] = blkv[:, None, :]
        xbias = xb.reshape(128, 4 * 128 * BL)
        c0T = np.ascontiguousarray(c0[bs].T)                # [H, BL]
        in_maps.append({
            "xT": xT.astype(np.float32),
            "xbias": xbias.astype(np.float32),
            "whT": whT.astype(np.float32),
            "wxT": wxT.astype(np.float32),
            "c0T": c0T.astype(np.float32),
            "wpredT": wpredT.astype(np.float32),
            "ebT": np.ascontiguousarray(ebT).astype(np.float32),
            "bpred": bpred_row,
        })

    nc = _get_program()
    res = bass_utils.run_bass_kernel_spmd(nc, in_maps,
                                          core_ids=list(range(NCORES)))

    out = np.empty((B, S, V), dtype=np.float32)
    for core in range(NCORES):
        oc = res.results[core]["out"]                       # [TOK, V] t-major
        out[core * BL:(core + 1) * BL] = oc.reshape(S, BL, V).transpose(1, 0, 2)
    return out
